# revision 50
# baseline (speedup 1.0000x reference)
"""Trainium2 Bass kernel for the MDL_RNN_mnist spiking network (v3).

Data-parallel over batch on 8 NeuronCores (BL = 64 per core).

Key structure (per step t, state layout [128, HT=16, BL=64]):
  - fp8e4 DoubleRow prepass accumulates input currents into the step's PSUM
    x-tile two steps ahead; exact lambda = 64 power-of-2 scaling throughout
    (winT8 = 2^15*(1-lm)*Win, spikes 2^-9, pinT8/u8 carry 8x, mem fp16 at
    64x with threshold 64).  lm*mem decay is a PE identity matmul into the
    same PSUM tile; the pin-stage matmul holds the stop.
  - NO separate refractory state: spikes are >= 3 steps apart, so the
    h-filter state itself classifies refractoriness exactly -- a 1-2 step
    old spike contributes >= 2*lr = 1.213 to HS while any legal older
    history sums to <= 2*lr^2/(1-lr^3) = 0.947.  mem = (HS<=1.08)*x is one
    fused DVE STT on the PSUM tile (saves a full-width op per step vs the
    old Ssum bookkeeping).  The reference's tlast=-1 init zeroes steps 0-1,
    so mask/spike/filter ops only run from t=2.
  - Rotated schedule: per iteration the PE does [decay_t, pin_t(stop),
    u(r of t), prepass x_{t+2}]; the DVE chain is [RMS(prev r), mem, s2,
    HS, RS] with the spike feeding the filter adds in the same iteration;
    Act runs [u8, HSp, RSp] (scale-copies, all ready early).  mem is
    double-buffered to kill the decay-read WAR.
  - Engine splits (cfg): the spike TS and the filter adds put their tail
    h-tiles on Pool (GPSIMD supports TS/add but NOT max/STT/PSUM access);
    the RMS max stays on DVE.
Final softmax (512x10) runs on host in fp32.
"""

import numpy as np
import ml_dtypes

T, DT, TAUM, TAUD, TAUR, VTHR, TREF = 100, 1.0, 10.0, 30.0, 2.0, 1.0, 2.0
B, IN, H, P, OUT = 512, 784, 2048, 256, 10
NCORES = 8
BL = B // NCORES          # 64 batch per core
HT, PT = H // 128, P // 128
INP8 = 1024               # input dim padded for fp8 DoubleRow (8 k-tiles)
KP = INP8 // 256          # 4 DoubleRow pairs
NCH = T // 2              # spike DMA chunks (2 steps each)
LAM = 64.0                # global x/mem scale (exact power of 2)
SPK_VAL = 2.0 ** -9       # spike value in fp8 (exact subnormal)
WIN_SC = 2.0 ** 15        # winT8 scale: 2^15 * spk 2^-9 = lam = 2^6
U_SC = 8.0                # u8 = 8*u ; pinT8 = 8*pin2 ; 8*8 = lam

BF16 = ml_dtypes.bfloat16
F8 = ml_dtypes.float8_e4m3fn

_CACHE = {}

# *_dve: how many of the 16 h-tiles of each op run on DVE (rest Pool)
_CFG = dict(s2_pool=6, hs_pool=6, rs_pool=6, mem_split=0, u8_eng="act", mask="theta", nsteps=T)


def _consts():
    lm = np.float32(np.exp(np.float32(-DT / TAUM)))
    ld = np.float32(np.exp(np.float32(-DT / TAUD)))
    lr = np.float32(np.exp(np.float32(-DT / TAUR)))
    c = np.float32(DT / (TAUD * TAUR))
    return lm, ld, lr, c


def _build_nc():
    import concourse.bacc as bacc
    import concourse.mybir as mybir
    import concourse.tile as tile

    dt = mybir.dt
    A = mybir.AluOpType
    AF = mybir.ActivationFunctionType
    PM = mybir.MatmulPerfMode
    lm, ld, lr, _c = _consts()
    ld, lr = float(ld), float(lr)
    cfg = _CFG
    NS = cfg["nsteps"]

    nc = bacc.Bacc("TRN2", target_bir_lowering=False, debug=False,
                   enable_asserts=False, num_devices=NCORES)

    spk_d = nc.dram_tensor("spk", [128, NCH, 2, 2 * KP, BL], dt.float8e4,
                           kind="ExternalInput").ap()
    poutT_d = nc.dram_tensor("poutT", [128, HT, PT, 128], dt.bfloat16,
                             kind="ExternalInput").ap()
    pinT8_d = nc.dram_tensor("pinT8", [128, PT, HT, 128], dt.float8e4,
                             kind="ExternalInput").ap()
    winT8_d = nc.dram_tensor("winT8", [128, KP, 2, HT, 128], dt.float8e4,
                             kind="ExternalInput").ap()
    idlm_d = nc.dram_tensor("idlm", [128, 128], dt.float16,
                            kind="ExternalInput").ap()
    woutT_d = nc.dram_tensor("woutT", [128, HT, OUT], dt.bfloat16,
                             kind="ExternalInput").ap()
    rout_d = nc.dram_tensor("rout", [BL, OUT], dt.float32,
                            kind="ExternalOutput").ap()

    def sbuf(name, shape, dtype):
        return nc.alloc_sbuf_tensor(name, list(shape), dtype).ap()

    # resident weights
    poutT = sbuf("poutT_sb", [128, HT, PT, 128], dt.bfloat16)
    pinT8 = sbuf("pinT8_sb", [128, PT, HT, 128], dt.float8e4)
    winT8 = sbuf("winT8_sb", [128, KP, 2, HT, 128], dt.float8e4)
    idlm = sbuf("idlm_sb", [128, 128], dt.float16)
    woutT = sbuf("woutT_sb", [128, HT, OUT], dt.bfloat16)

    # persistent state [128, HT, BL]
    mem = [sbuf("mem_a", [128, HT, BL], dt.float16),  # masked membrane * lam
           sbuf("mem_b", [128, HT, BL], dt.float16)]  # (ring 2: no WAR vs decay)
    HS = sbuf("HS", [128, HT, BL], dt.float16)        # (2/c) * h
    RMS = sbuf("RMS", [128, HT, BL], dt.float16)      # (2/c) * max_t r
    HSp = sbuf("HSp", [128, HT, BL], dt.float16)      # scratch lr*HS
    RSp = sbuf("RSp", [128, HT, BL], dt.float16)      # scratch ld*RS
    RS = [sbuf("RS_a", [128, HT, BL], dt.float16),    # (2/c) * r, ring 2
          sbuf("RS_b", [128, HT, BL], dt.float16)]
    s2 = sbuf("s2", [128, HT, BL], dt.float16)        # 2 * spike
    s2p_prev = sbuf("s2_prev", [128, HT, BL], dt.float16)  # (ssum mask mode)
    Ssum = sbuf("Ssum", [128, HT, BL], dt.float16)         # (ssum mask mode)
    # Refractory mask comes from HS itself: a spike 1-2 steps old contributes
    # >= 2*lr = 1.213 to HS, while any older legal (gap>=3) spike pattern
    # sums to <= 2*lr^2/(1-lr^3) = 0.947.  So (HS <= THETA) <=> not
    # refractory, exactly.  No separate spike-history state needed.
    THETA = 1.08

    with tile.TileContext(nc, trace_sim=False) as tc:
        nc.sync.dma_start(poutT, poutT_d)
        nc.sync.dma_start(pinT8, pinT8_d)
        nc.sync.dma_start(winT8, winT8_d)
        nc.sync.dma_start(idlm, idlm_d)
        nc.sync.dma_start(woutT, woutT_d)

        nc.vector.memset(mem[0], 0.0)   # mem_0 = mem_1 = 0: the reference
        nc.vector.memset(mem[1], 0.0)   # tlast=-1 init masks steps 0 and 1
        nc.vector.memset(HS, 0.0)
        nc.vector.memset(RMS, 0.0)
        nc.vector.memset(RS[0], 0.0)
        nc.vector.memset(RS[1], 0.0)
        nc.vector.memset(HSp, 0.0)
        nc.vector.memset(RSp, 0.0)
        if cfg["mask"] == "ssum":
            nc.vector.memset(s2p_prev, 0.0)
            nc.vector.memset(Ssum, 0.0)

        with tc.tile_pool(name="sp", bufs=3) as sp_pool, \
             tc.tile_pool(name="u8", bufs=2) as u8_pool, \
             tc.tile_pool(name="x_ps", bufs=3, space="PSUM") as x_pool, \
             tc.tile_pool(name="u_ps", bufs=2, space="PSUM") as u_pool:

            def dma_chunk(c):
                sp = sp_pool.tile([128, 2, 2 * KP, BL], dt.float8e4, tag="sp")
                nc.sync.dma_start(sp, spk_d[:, c, :, :, :])
                return sp

            def emit_prepass(x_t, sp, tin, last):
                # 64 fp8 DoubleRow matmuls: x_t[:, i, :] = sum winT8 @ spk
                for i in range(HT):
                    for kp in range(KP):
                        nc.tensor.matmul(
                            x_t[:, i, :], winT8[:, kp, :, i, :],
                            sp[:, tin, 2 * kp:2 * kp + 2, :],
                            start=(kp == 0), stop=(last and kp == KP - 1),
                            perf_mode=PM.DoubleRow)

            # Steps 0 and 1 have all-zero dynamics (tlast=-1 refractory):
            # mem_0 = mem_1 = 0, s_0 = s_1 = 0, r^1..r^3 = 0, so mask/spike/
            # filter ops start at t=2; matmuls run from t=0 on zero states.
            sp_cur = dma_chunk(0)
            sp_next = dma_chunk(1)
            x_tiles = {}
            for tau in (0, 1):
                x_tiles[tau] = x_pool.tile([128, HT, BL], dt.float32,
                                           tag="x", name=f"x_{tau}")
                emit_prepass(x_tiles[tau], sp_cur, tau % 2, last=(tau == 0))

            u8_cur = None
            u_defer = None
            mask_mode = cfg["mask"]
            s2p = cfg["s2_pool"]
            hp, rp = cfg["hs_pool"], cfg["rs_pool"]
            hd, rd_ = HT - hp, HT - rp
            msp = cfg["mem_split"]
            u8e = cfg["u8_eng"]
            # Rotated schedule. Iteration t emits:
            #   PE : decay_t (mem_{t-1}), pin_t (u8 of r^t), u-matmul of
            #        r^{t+1} (RS written last iter), prepass x_{t+2}
            #   DVE: s2_t -> HS (h^{t+2}) -> RS (r^{t+2}) with zero queue gap
            #   Act: u8(r^{t+1}), HSp, RSp
            #   Pool: mem_t (PSUM read), tail tiles of RMS
            for t in range(NS):
                RS_w = RS[t % 2]            # r^{t+2}, written this iter
                RS_r = RS[(t + 1) % 2]      # r^{t+1}, written last iter
                x_t = x_tiles.pop(t)
                mem_prev, mem_cur = mem[(t + 1) % 2], mem[t % 2]

                # ---- PE: finish x_t. decay (mem_prev); pin stage (u8_cur)
                # holds the stop except at t=1 (no pin yet). ----
                if t > 0:
                    if t > 1:
                        for i in range(HT):
                            nc.tensor.matmul(x_t[:, i, :], pinT8[:, :, i, :],
                                             u8_cur, start=False, stop=False,
                                             perf_mode=PM.DoubleRow)
                    for i in range(HT):
                        nc.tensor.matmul(x_t[:, i, :], idlm,
                                         mem_prev[:, i, :],
                                         start=False, stop=True)
                # ---- PE: u = pout @ r (for pin at t+1); zero states early ----
                if 0 < t < NS - 1:
                    u_ps = u_pool.tile([128, PT, BL], dt.float32, tag="u")
                    for q in range(PT):
                        for j in range(HT):
                            nc.tensor.matmul(u_ps[:, q, :], poutT[:, j, q, :],
                                             RS_r[:, j, :],
                                             start=(j == 0),
                                             stop=(j == HT - 1))
                    u8_cur = u8_pool.tile([128, PT, BL], dt.float8e4,
                                          tag="u8")
                    if u8e == "act":
                        nc.scalar.activation(u8_cur, u_ps, AF.Copy)
                        u_defer = None
                    else:
                        u_defer = (u8_cur, u_ps)

                # ---- DVE: mask via HS refractory threshold (PSUM read;
                # GPSIMD cannot touch PSUM).  HS here is h^{t}. ----
                if t < 2:
                    # steps 0,1: all state stays zero; only emit the prepass
                    if u8e == "dve" and u_defer is not None:
                        nc.vector.tensor_copy(u_defer[0], u_defer[1])
                        u_defer = None
                    tau = t + 2
                    if tau % 2 == 0:
                        sp_cur = sp_next
                        if tau // 2 + 1 < NCH:
                            sp_next = dma_chunk(tau // 2 + 1)
                    x_tiles[tau] = x_pool.tile([128, HT, BL], dt.float32,
                                               tag="x", name=f"x_{tau}")
                    emit_prepass(x_tiles[tau], sp_cur, tau % 2, last=False)
                    continue
                # ---- RMS vs last iter's r (ready now; fills the gap
                # while x_t finishes).  Covers r^2..r^99 over all iters. ----
                nc.vector.tensor_tensor(RMS, RMS, RS_r, op=A.max)
                if mask_mode == "theta":
                    if msp:
                        nc.vector.scalar_tensor_tensor(
                            mem_cur[:, :msp, :], HS[:, :msp, :], THETA,
                            x_t[:, :msp, :], op0=A.is_le, op1=A.mult)
                        nc.vector.scalar_tensor_tensor(
                            mem_cur[:, msp:, :], HS[:, msp:, :], THETA,
                            x_t[:, msp:, :], op0=A.is_le, op1=A.mult)
                    else:
                        nc.vector.scalar_tensor_tensor(mem_cur, HS, THETA,
                                                       x_t, op0=A.is_le,
                                                       op1=A.mult)
                else:
                    nc.vector.scalar_tensor_tensor(mem_cur, Ssum, 0.0, x_t,
                                                   op0=A.is_equal, op1=A.mult)
                # ---- spike: DVE low tiles (4x TS), Pool high tiles ----
                sp_hi = HT - s2p
                nc.vector.tensor_scalar(s2[:, :sp_hi, :],
                                        mem_cur[:, :sp_hi, :],
                                        float(LAM * VTHR), 2.0,
                                        op0=A.is_gt, op1=A.mult)
                if s2p:
                    nc.gpsimd.tensor_scalar(s2[:, sp_hi:, :],
                                            mem_cur[:, sp_hi:, :],
                                            float(LAM * VTHR), 2.0,
                                            op0=A.is_gt, op1=A.mult)
                if u8e == "dve" and u_defer is not None:
                    nc.vector.tensor_copy(u_defer[0], u_defer[1])
                    u_defer = None
                if mask_mode == "ssum":
                    nc.vector.tensor_tensor(Ssum, s2, s2p_prev, op=A.add)
                    nc.vector.tensor_copy(s2p_prev, s2)
                if t < NS - 1:
                    if hp:
                        nc.gpsimd.tensor_tensor(HS[:, hd:, :], HSp[:, hd:, :],
                                                s2[:, hd:, :], op=A.add)
                    nc.vector.tensor_tensor(HS[:, :hd, :], HSp[:, :hd, :],
                                            s2[:, :hd, :], op=A.add)
                    if rp:
                        nc.gpsimd.tensor_tensor(RS_w[:, rd_:, :],
                                                RSp[:, rd_:, :],
                                                HS[:, rd_:, :], op=A.add)
                    nc.vector.tensor_tensor(RS_w[:, :rd_, :], RSp[:, :rd_, :],
                                            HS[:, :rd_, :], op=A.add)
                    nc.scalar.activation(HSp, HS, AF.Copy, scale=lr)
                    nc.scalar.activation(RSp, RS_w, AF.Copy, scale=ld)

                # ---- PE: prepass for step t+2 (fills PE while mem runs) ----
                tau = t + 2
                if tau < NS:
                    if tau % 2 == 0:
                        sp_cur = sp_next
                        if tau // 2 + 1 < NCH:
                            sp_next = dma_chunk(tau // 2 + 1)
                    x_tiles[tau] = x_pool.tile([128, HT, BL], dt.float32,
                                               tag="x", name=f"x_{tau}")
                    emit_prepass(x_tiles[tau], sp_cur, tau % 2, last=False)

        # ---- readout: rout = RMS @ woutT ----
        with tc.tile_pool(name="rp", bufs=1, space="PSUM") as rp_pool, \
             tc.tile_pool(name="ro", bufs=1) as ro_pool:
            rp = rp_pool.tile([BL, OUT], dt.float32, tag="rp")
            for j in range(HT):
                nc.tensor.matmul(rp, RMS[:, j, :], woutT[:, j, :],
                                 start=(j == 0), stop=(j == HT - 1))
            ro = ro_pool.tile([BL, OUT], dt.float32, tag="ro")
            nc.vector.tensor_copy(ro, rp)
            nc.sync.dma_start(rout_d, ro)

    nc.compile()
    return nc


def _get_nc():
    if "nc" not in _CACHE:
        _CACHE["nc"] = _build_nc()
    return _CACHE["nc"]


def _make_spikes(inputs):
    """Bit-exact reference spikes: bernoulli(key(42), inputs, (T,B,IN))."""
    import jax
    cpu = jax.devices("cpu")[0]
    with jax.default_device(cpu):
        spk = jax.random.bernoulli(
            jax.random.key(42), jax.numpy.asarray(inputs), shape=(T, B, IN))
        return np.asarray(spk)


def prepare_in_maps(inputs, Win, pin, pout, l, Wout):
    lm, ld, lr, c = _consts()
    one_m_lm = np.float32(1.0) - lm

    # poutT[p, j, q, pp] = 8*(c/2)*pout[j*128+p, q*128+pp]   (bf16)
    pout2 = (np.float32(U_SC) * (np.float32(0.5) * c) * pout).astype(np.float32)
    poutT = np.ascontiguousarray(
        pout2.reshape(HT, 128, PT, 128).transpose(1, 0, 2, 3)).astype(BF16)
    # pinT8[pp, q, i, p] = 8*(1-lm)*(l*pin)[i*128+p, q*128+pp]   (e4m3)
    pin2 = (np.float32(U_SC) * one_m_lm * (l * pin)).astype(np.float32)
    pinT8 = np.ascontiguousarray(
        pin2.reshape(HT, 128, PT, 128).transpose(3, 2, 0, 1)).astype(F8)
    # winT8[ik, kp, j, ih, hp] = 2^15*(1-lm)*Win[ih*128+hp, (2kp+j)*128+ik]
    winp = np.zeros((H, INP8), np.float32)
    winp[:, :IN] = np.float32(WIN_SC) * one_m_lm * Win
    winT8 = np.ascontiguousarray(
        winp.reshape(HT, 128, KP, 2, 128).transpose(4, 2, 3, 0, 1)).astype(F8)
    idlm = (np.float16(lm) * np.eye(128)).astype(np.float16)
    # woutT[hp, j, o] = (c/2)*Wout[o, j*128+hp]   (bf16)
    wout2 = (np.float32(0.5) * c * Wout).astype(np.float32)
    woutT = np.ascontiguousarray(
        wout2.T.reshape(HT, 128, OUT).transpose(1, 0, 2)).astype(BF16)

    spk = _make_spikes(inputs)                          # (T, B, IN) bool
    # sp8[ik, c, tin, kt, b] = 2^-9 * spk[2c+tin, b, kt*128+ik]
    sp = np.zeros((INP8, T, B), np.float32)
    sp[:IN] = spk.transpose(2, 0, 1).astype(np.float32) * np.float32(SPK_VAL)
    sp = sp.reshape(2 * KP, 128, NCH, 2, B).transpose(1, 2, 3, 0, 4)
    sp8 = sp.astype(F8)                                 # [128, NCH, 2, 8, B]

    in_maps = []
    for cid in range(NCORES):
        in_maps.append({
            "spk": np.ascontiguousarray(sp8[:, :, :, :,
                                            cid * BL:(cid + 1) * BL]),
            "poutT": poutT,
            "pinT8": pinT8,
            "winT8": winT8,
            "idlm": idlm,
            "woutT": woutT,
        })
    return in_maps


def run_device(nc, in_maps):
    from concourse.bass_utils import run_bass_kernel_spmd
    res = run_bass_kernel_spmd(nc, in_maps, list(range(NCORES)))
    return np.concatenate([res.results[cid]["rout"] for cid in range(NCORES)],
                          axis=0)


def _softmax32(x):
    e = np.exp(x - x.max(axis=1, keepdims=True), dtype=np.float32)
    return (e / e.sum(axis=1, keepdims=True,
                      dtype=np.float32)).astype(np.float32)


def kernel(inputs, Win, pin, pout, l, Wout):
    inputs = np.asarray(inputs, np.float32)
    Win = np.asarray(Win, np.float32)
    pin = np.asarray(pin, np.float32)
    pout = np.asarray(pout, np.float32)
    l = np.asarray(l, np.float32)
    Wout = np.asarray(Wout, np.float32)

    nc = _get_nc()
    in_maps = prepare_in_maps(inputs, Win, pin, pout, l, Wout)
    rout = run_device(nc, in_maps)                      # (512, 10) fp32
    return _softmax32(rout)

